# revision 13
# baseline (speedup 1.0000x reference)
"""Multi-head attention (B=2, S=2048, D=1024, H=16) on 8 Trainium2 NeuronCores.

Sharding: tensor-parallel over heads x data-parallel over batch.
  core c -> batch g = c // 4, head group r = c % 4 (global heads 4r..4r+3).
Each core computes qkv for its 4 heads (two head pairs), attention over the
full sequence of its batch, and a PARTIAL output projection over its own 256
head-dims for ALL 2048 rows.  The four partials per batch are summed on the
host (plus b_proj) -- no device collectives at all.

Device schedule (emission order == per-engine program order):
  phase A: stream xT in k-tiles, qk(pair0) k-major into 8 PSUM banks,
           then v(pair0) m-major.
  phase B: for pair p, for cq (512-query chunk), for t (128-key tile):
             scores: two row-packed K=64 matmuls (head 2p rows 0-63,
                     head 2p+1 rows 64-127) into one [128,1024] PSUM tile
                     (two banks, no write conflict),
             ONE exp ACT over [128,1024] (both heads) -> bf16,
             two AV matmuls accumulating [65,512] (ones column of V picks
                     up the softmax denominator).
           qkv(pair1) is emitted in small pieces between t-iterations of
           pair0's attention; projection chunks likewise ride inside
           pair1's attention.  The scalar engine (exp) is the critical
           resource; the PE fills its shadow.
  normalize per (p,cq): rowsum bcast via tiny K=1 matmul, reciprocal and
           multiply on DVE -> aoT bf16.
  proj: per s-tile: psum[128,512] = aoT[0].T @ wp[0] + aoT[1].T @ wp[1],
           copied to fp32 and DMA'd out (partial, host sums).
"""

import os
import sys

import numpy as np

try:
    import ml_dtypes
    BF16_NP = ml_dtypes.bfloat16
except ImportError:  # pragma: no cover
    BF16_NP = None

for _p in ("/opt/trn_rl_repo",):
    if os.path.isdir(_p) and _p not in sys.path:
        sys.path.append(_p)

import concourse.bass as bass  # noqa: E402
import concourse.mybir as mybir  # noqa: E402
import concourse.tile as tile  # noqa: E402
from concourse import bacc  # noqa: E402
from concourse.bass_utils import run_bass_kernel_spmd  # noqa: E402

B, S, D = 2, 2048, 1024
H, HD = 16, 64
N_CORES = 8
GROUP = 4           # cores per batch group
LH = H // GROUP     # local heads per core = 4 (2 pairs)
LHD = LH * HD       # 256 local head dims
FP32 = mybir.dt.float32
FP16 = mybir.dt.float16
FP32R = mybir.dt.float32r
BF16 = mybir.dt.bfloat16

SQ = 512            # query chunk
N_CQ = S // SQ      # 4
N_SK = S // 128     # 16 key tiles
N_KT = D // 128     # 8 contraction tiles

_compiled = None
_ONES = np.ones((1, 64), dtype=np.float32)


def _build():
    nc = bacc.Bacc(
        "TRN2", target_bir_lowering=False, debug=False, num_devices=N_CORES
    )

    xT_d = nc.dram_tensor("xT", [D, S], BF16, kind="ExternalInput")
    wq_d = nc.dram_tensor("wq", [D, LHD], BF16, kind="ExternalInput")
    wk_d = nc.dram_tensor("wk", [D, LHD], BF16, kind="ExternalInput")
    wv_d = nc.dram_tensor("wv", [D, LHD], BF16, kind="ExternalInput")
    wp_d = nc.dram_tensor("wp", [LHD, D], BF16, kind="ExternalInput")
    ones_d = nc.dram_tensor("ones", [1, 64], FP32R, kind="ExternalInput")
    bq_d = nc.dram_tensor("bq", [LHD, 1], FP32, kind="ExternalInput")
    bk_d = nc.dram_tensor("bk", [LHD, 1], FP32, kind="ExternalInput")
    bv_d = nc.dram_tensor("bv", [128, LHD], FP32, kind="ExternalInput")
    out_d = nc.dram_tensor("out", [S, D], FP16, kind="ExternalOutput")

    with tile.TileContext(nc) as tc:
        import contextlib

        with contextlib.ExitStack() as stk:
            # ---- long-lived SBUF pools --------------------------------
            qk_pool = stk.enter_context(tc.tile_pool(name="qk", bufs=1))
            v_pool = stk.enter_context(tc.tile_pool(name="v", bufs=1))
            ao_pool = stk.enter_context(tc.tile_pool(name="ao", bufs=1))
            const_pool = stk.enter_context(tc.tile_pool(name="const", bufs=1))
            w_pool = stk.enter_context(tc.tile_pool(name="w", bufs=1))
            x_pool = stk.enter_context(tc.tile_pool(name="x", bufs=1))

            qT = [qk_pool.tile([128, S], BF16, name=f"qT{p}", tag=f"qT{p}")
                  for p in range(2)]
            kT = [qk_pool.tile([128, S], BF16, name=f"kT{p}", tag=f"kT{p}")
                  for p in range(2)]
            # vp[p][m]: [128 keys, 130] = head2p v | 1.0 | head2p+1 v | 1.0
            vp = [[v_pool.tile([128, 130], BF16, name=f"v{p}_{m}",
                               tag=f"v{p}_{m}") for m in range(N_SK)]
                  for p in range(2)]
            aoT = [ao_pool.tile([128, S], BF16, name=f"ao{p}", tag=f"ao{p}")
                   for p in range(2)]

            ones_t = const_pool.tile([1, 64], FP32R, tag="ones")
            nc.sync.dma_start(ones_t[:], ones_d.ap())
            bq_t = [const_pool.tile([128, 1], FP32, name=f"bq{p}",
                                    tag=f"bq{p}") for p in range(2)]
            bk_t = [const_pool.tile([128, 1], FP32, name=f"bk{p}",
                                    tag=f"bk{p}") for p in range(2)]
            bv_t = const_pool.tile([128, LHD], FP32, tag="bv")
            for p in range(2):
                psl = slice(p * 128, (p + 1) * 128)
                nc.sync.dma_start(bq_t[p][:], bq_d.ap()[psl, :])
                nc.sync.dma_start(bk_t[p][:], bk_d.ap()[psl, :])
            nc.sync.dma_start(bv_t[:], bv_d.ap())

            x_t = [[x_pool.tile([128, SQ], BF16, name=f"x{k}_{sc}",
                                tag=f"x{k}_{sc}") for sc in range(4)]
                   for k in range(N_KT)]
            wq_t = [w_pool.tile([128, LHD], BF16, name=f"wq{k}", tag=f"wq{k}")
                    for k in range(N_KT)]
            wk_t = [w_pool.tile([128, LHD], BF16, name=f"wk{k}", tag=f"wk{k}")
                    for k in range(N_KT)]
            wv_t = [w_pool.tile([128, LHD], BF16, name=f"wv{k}", tag=f"wv{k}")
                    for k in range(N_KT)]
            wp_t = [w_pool.tile([128, D], BF16, name=f"wp{p}", tag=f"wp{p}")
                    for p in range(2)]

            # input DMA: x on sync queue, weights on scalar queue (scalar
            # engine is idle during phase A)
            def xdma(k, sc):
                sl = slice(k * 128, (k + 1) * 128)
                ssl = slice(sc * SQ, (sc + 1) * SQ)
                eng = nc.sync if k % 2 == 0 else nc.gpsimd
                eng.dma_start(x_t[k][sc][:], xT_d.ap()[sl, ssl])

            def wdma(w_t, w_d, k):
                sl = slice(k * 128, (k + 1) * 128)
                eng = nc.sync if k % 2 == 0 else nc.gpsimd
                eng.dma_start(w_t[k][:], w_d.ap()[sl, :])

            for k in range(N_KT):
                xdma(k, 0)
            for k in range(N_KT):
                wdma(wk_t, wk_d, k)
            for k in range(N_KT):
                wdma(wq_t, wq_d, k)
            for k in range(N_KT):
                wdma(wv_t, wv_d, k)
            for sc in range(1, 4):
                for k in range(N_KT):
                    xdma(k, sc)
            for p in range(2):
                nc.scalar.dma_start(wp_t[p][:],
                                    wp_d.ap()[p * 128:(p + 1) * 128, :])

            # ---- phase B pools ----------------------------------------
            sc_pool = stk.enter_context(
                tc.tile_pool(name="sc", bufs=2, space="PSUM"))
            acc_pool = stk.enter_context(
                tc.tile_pool(name="acc", bufs=1, space="PSUM"))
            misc_pool = stk.enter_context(
                tc.tile_pool(name="misc", bufs=2, space="PSUM"))
            p_pool = stk.enter_context(tc.tile_pool(name="pt", bufs=6))
            rr_pool = stk.enter_context(tc.tile_pool(name="rr", bufs=2))
            rc_pool = stk.enter_context(tc.tile_pool(name="rc", bufs=2))
            ost_pool = stk.enter_context(tc.tile_pool(name="ost", bufs=2))

            # ---- phase-A chunk emitters (run through the misc ring) ----
            def qk0_chunk(which, sc):
                w_t, b_t, dstT = ((wq_t, bq_t[0], qT[0]) if which == "q"
                                  else (wk_t, bk_t[0], kT[0]))
                ssl = slice(sc * SQ, (sc + 1) * SQ)

                def emit():
                    ps = misc_pool.tile([128, SQ], FP32,
                                        name=f"m0{which}{sc}", tag="m")
                    for k in range(N_KT):
                        nc.tensor.matmul(
                            ps[:], w_t[k][:, 0:128], x_t[k][sc][:],
                            start=(k == 0), stop=(k == N_KT - 1),
                        )
                    nc.vector.tensor_scalar(
                        dstT[:, ssl], ps[:], b_t[:], None,
                        mybir.AluOpType.add,
                    )
                return emit

            def v_chunk(m):
                def emit():
                    ps = misc_pool.tile([128, SQ], FP32,
                                        name=f"m0v{m}", tag="m")
                    xo = (m % 4) * 128
                    for k in range(N_KT):
                        nc.tensor.matmul(
                            ps[:, 0:LHD],
                            x_t[k][m // 4][:, xo:xo + 128], wv_t[k][:],
                            start=(k == 0), stop=(k == N_KT - 1),
                        )
                    for p in range(2):
                        nc.vector.tensor_tensor(
                            vp[p][m][:, 0:64], ps[:, p * 128:p * 128 + 64],
                            bv_t[:, p * 128:p * 128 + 64],
                            mybir.AluOpType.add,
                        )
                        nc.vector.tensor_tensor(
                            vp[p][m][:, 65:129],
                            ps[:, p * 128 + 64:p * 128 + 128],
                            bv_t[:, p * 128 + 64:p * 128 + 128],
                            mybir.AluOpType.add,
                        )
                        nc.vector.memset(vp[p][m][:, 64::65], 1.0)
                return emit

            # ---- deferred emission units (PE filler work) -------------
            filler = []

            def qk1_chunk_parts(which, sc):
                # q/k pair-1 chunk: 8 accumulating MMs + DVE drain, split
                # into 2-MM pieces so injection granularity stays ~0.5us
                w_t, b_t, dstT = ((wq_t, bq_t[1], qT[1]) if which == "q"
                                  else (wk_t, bk_t[1], kT[1]))
                ssl = slice(sc * SQ, (sc + 1) * SQ)
                state = {}

                def piece(k0):
                    def emit():
                        if k0 == 0:
                            state["ps"] = misc_pool.tile(
                                [128, SQ], FP32,
                                name=f"mqk{which}{sc}", tag="m")
                        ps = state["ps"]
                        for k in (k0, k0 + 1):
                            nc.tensor.matmul(
                                ps[:], w_t[k][:, 128:256], x_t[k][sc][:],
                                start=(k == 0), stop=(k == N_KT - 1),
                            )
                        if k0 == N_KT - 2:
                            nc.vector.tensor_scalar(
                                dstT[:, ssl], ps[:], b_t[:], None,
                                mybir.AluOpType.add,
                            )
                    return emit
                return [piece(k0) for k0 in range(0, N_KT, 2)]

            for sc in range(4):
                filler.extend(qk1_chunk_parts("q", sc))
                filler.extend(qk1_chunk_parts("k", sc))

            def proj_unit(m, nb, tail=False):
                # partial projection for s-tile m, dout half nb
                msl = slice(m * 128, (m + 1) * 128)
                nsl = slice(nb * SQ, (nb + 1) * SQ)

                def emit():
                    ps = misc_pool.tile([128, SQ], FP32,
                                        name=f"mpj{m}_{nb}", tag="m")
                    for p in range(2):
                        nc.tensor.matmul(
                            ps[:], aoT[p][:, msl], wp_t[p][:, nsl],
                            start=(p == 0), stop=(p == 1),
                        )
                    ot = ost_pool.tile([128, SQ], FP16,
                                       name=f"ot{m}_{nb}", tag="ot")
                    if tail and (m + nb) % 2 == 0:
                        nc.scalar.copy(ot[:], ps[:])
                    else:
                        nc.vector.tensor_copy(ot[:], ps[:])
                    eng = nc.gpsimd if (m + nb) % 2 == 0 else nc.sync
                    eng.dma_start(out_d.ap()[msl, nsl], ot[:])
                return emit

            def inject(n):
                for _ in range(n):
                    if filler:
                        filler.pop(0)()

            # ---- phase B: attention -----------------------------------
            for p in range(2):
                for cq in range(N_CQ):
                    qsl = slice(cq * SQ, (cq + 1) * SQ)
                    acc_a = acc_pool.tile([65, SQ], FP32, tag="acca")
                    acc_b = acc_pool.tile([65, SQ], FP32, tag="accb")
                    for t in range(N_SK):
                        if p == 0 and cq == 0 and t % 4 == 0:
                            # qk(p0) for key/query column chunk t//4,
                            # gated on x pieces as they stream in
                            sc = t // 4
                            qk0_chunk("k", sc)()
                            if sc == 0:
                                qk0_chunk("q", sc)()
                        tsl = slice(t * 128, (t + 1) * 128)
                        sc_ab = sc_pool.tile([128, 2 * SQ], FP32, tag="sc")
                        nc.tensor.matmul(
                            sc_ab[:, 0:SQ], kT[p][0:64, tsl],
                            qT[p][0:64, qsl],
                            start=True, stop=True, tile_position=(0, 0),
                        )
                        nc.tensor.matmul(
                            sc_ab[:, SQ:2 * SQ], kT[p][64:128, tsl],
                            qT[p][64:128, qsl],
                            start=True, stop=True, tile_position=(64, 0),
                        )
                        pab = p_pool.tile([128, 2 * SQ], BF16, tag="pt")
                        nc.scalar.activation(
                            pab[:], sc_ab[:],
                            mybir.ActivationFunctionType.Exp, scale=0.125,
                        )
                        if p == 0 and cq == 0:
                            v_chunk(t)()
                            if t % 4 == 3 and t < 12:
                                qk0_chunk("q", t // 4 + 1)()
                        nc.tensor.matmul(
                            acc_a[:], vp[p][t][:, 0:65], pab[:, 0:SQ],
                            start=(t == 0), stop=(t == N_SK - 1),
                        )
                        nc.tensor.matmul(
                            acc_b[:], vp[p][t][:, 65:130], pab[:, SQ:2 * SQ],
                            start=(t == 0), stop=(t == N_SK - 1),
                        )
                        if not (p == 0 and cq == 0):
                            inject(2 if p == 0 else 1)
                    # normalize both heads of this (p, cq)
                    for acc, half in ((acc_a, 0), (acc_b, 1)):
                        rrow = rr_pool.tile([1, SQ], FP32R, tag="rrow")
                        nc.vector.tensor_copy(rrow[:], acc[64:65, :])
                        rbt = misc_pool.tile([128, SQ], FP32, tag="m",
                                             name=f"rb{p}{cq}{half}")
                        nc.tensor.matmul(
                            rbt[0:64, :], ones_t[0:1, 0:64], rrow[:],
                            start=True, stop=True,
                        )
                        rc = rc_pool.tile([64, SQ], FP32, tag="rc")
                        nc.vector.reciprocal_approx_fast(rc[:], rbt[0:64, :])
                        nc.vector.tensor_tensor(
                            aoT[p][64 * half:64 * half + 64, qsl],
                            acc[0:64, :], rc[:],
                            mybir.AluOpType.mult,
                        )
                    if p == 1:
                        # projection for the chunk finished one cq ago
                        # rides inside the next chunk's t-loop via filler
                        tail = cq == N_CQ - 1
                        for m in range(cq * 4, cq * 4 + 4):
                            for nb in range(2):
                                filler.append(proj_unit(m, nb, tail))
            # flush remaining filler (last chunk's projection)
            while filler:
                filler.pop(0)()

    nc.compile()
    return nc


def _get_program():
    global _compiled
    if _compiled is None:
        _compiled = _build()
    return _compiled


def _make_in_maps(x, w_qkv, b_qkv, w_proj, b_proj):
    x = np.asarray(x, dtype=np.float32)
    w_qkv = np.asarray(w_qkv, dtype=np.float32)
    b_qkv = np.asarray(b_qkv, dtype=np.float32)
    w_proj = np.asarray(w_proj, dtype=np.float32)

    in_maps = []
    for c in range(N_CORES):
        g, r = c // GROUP, c % GROUP
        xT = np.ascontiguousarray(x[g].T)
        csl = slice(r * LHD, (r + 1) * LHD)
        in_maps.append(
            {
                "xT": xT.astype(BF16_NP),
                "wq": w_qkv[:, 0 * D + r * LHD:0 * D + (r + 1) * LHD].astype(BF16_NP),
                "wk": w_qkv[:, 1 * D + r * LHD:1 * D + (r + 1) * LHD].astype(BF16_NP),
                "wv": w_qkv[:, 2 * D + r * LHD:2 * D + (r + 1) * LHD].astype(BF16_NP),
                "wp": np.ascontiguousarray(w_proj[csl, :]).astype(BF16_NP),
                "bq": np.ascontiguousarray(
                    b_qkv[0 * D + r * LHD:0 * D + (r + 1) * LHD].reshape(LHD, 1)),
                "bk": np.ascontiguousarray(
                    b_qkv[1 * D + r * LHD:1 * D + (r + 1) * LHD].reshape(LHD, 1)),
                "ones": _ONES,
                "bv": np.ascontiguousarray(
                    np.broadcast_to(
                        b_qkv[2 * D + r * LHD:2 * D + (r + 1) * LHD].reshape(1, LHD),
                        (128, LHD),
                    )
                ),
            }
        )
    return in_maps


def _assemble(results, b_proj):
    out = np.empty((B, S, D), dtype=np.float32)
    for g in range(B):
        acc = results[g * GROUP]["out"].astype(np.float32)
        for r in range(1, GROUP):
            acc += results[g * GROUP + r]["out"].astype(np.float32)
        out[g] = acc + np.asarray(b_proj, dtype=np.float32).reshape(1, D)
    return out


def kernel(x, w_qkv, b_qkv, w_proj, b_proj):
    nc = _get_program()
    in_maps = _make_in_maps(x, w_qkv, b_qkv, w_proj, b_proj)
    res = run_bass_kernel_spmd(nc, in_maps, list(range(N_CORES)))
    return _assemble(res.results, b_proj)


# revision 15
# speedup vs baseline: 1.0083x; 1.0083x over previous
"""Multi-head attention (B=2, S=2048, D=1024, H=16) on 8 Trainium2 NeuronCores.

Sharding: tensor-parallel over heads x data-parallel over batch.
  core c -> batch g = c // 4, head group r = c % 4 (global heads 4r..4r+3).
Each core computes qkv for its 4 heads (two head pairs), attention over the
full sequence of its batch, and a PARTIAL output projection over its own 256
head-dims for ALL 2048 rows.  The four partials per batch are summed on the
host (plus b_proj) -- no device collectives at all.

Device schedule (emission order == per-engine program order):
  phase A: stream xT in k-tiles, qk(pair0) k-major into 8 PSUM banks,
           then v(pair0) m-major.
  phase B: for pair p, for cq (512-query chunk), for t (128-key tile):
             scores: two row-packed K=64 matmuls (head 2p rows 0-63,
                     head 2p+1 rows 64-127) into one [128,1024] PSUM tile
                     (two banks, no write conflict),
             ONE exp ACT over [128,1024] (both heads) -> bf16,
             two AV matmuls accumulating [65,512] (ones column of V picks
                     up the softmax denominator).
           qkv(pair1) is emitted in small pieces between t-iterations of
           pair0's attention; projection chunks likewise ride inside
           pair1's attention.  The scalar engine (exp) is the critical
           resource; the PE fills its shadow.
  normalize per (p,cq): rowsum bcast via tiny K=1 matmul, reciprocal and
           multiply on DVE -> aoT bf16.
  proj: per s-tile: psum[128,512] = aoT[0].T @ wp[0] + aoT[1].T @ wp[1],
           copied to fp32 and DMA'd out (partial, host sums).
"""

import os
import sys

import numpy as np

try:
    import ml_dtypes
    BF16_NP = ml_dtypes.bfloat16
except ImportError:  # pragma: no cover
    BF16_NP = None

for _p in ("/opt/trn_rl_repo",):
    if os.path.isdir(_p) and _p not in sys.path:
        sys.path.append(_p)

import concourse.bass as bass  # noqa: E402
import concourse.mybir as mybir  # noqa: E402
import concourse.tile as tile  # noqa: E402
from concourse import bacc  # noqa: E402
from concourse.bass_utils import run_bass_kernel_spmd  # noqa: E402

B, S, D = 2, 2048, 1024
H, HD = 16, 64
N_CORES = 8
GROUP = 4           # cores per batch group
LH = H // GROUP     # local heads per core = 4 (2 pairs)
LHD = LH * HD       # 256 local head dims
FP32 = mybir.dt.float32
FP16 = mybir.dt.float16
FP32R = mybir.dt.float32r
BF16 = mybir.dt.bfloat16

SQ = 512            # query chunk
N_CQ = S // SQ      # 4
N_SK = S // 128     # 16 key tiles
N_KT = D // 128     # 8 contraction tiles

_compiled = None
_ONES = np.ones((1, 64), dtype=np.float32)


def _build():
    nc = bacc.Bacc(
        "TRN2", target_bir_lowering=False, debug=False, num_devices=N_CORES
    )

    xT_d = nc.dram_tensor("xT", [D, S], BF16, kind="ExternalInput")
    wq_d = nc.dram_tensor("wq", [D, LHD], BF16, kind="ExternalInput")
    wk_d = nc.dram_tensor("wk", [D, LHD], BF16, kind="ExternalInput")
    wv_d = nc.dram_tensor("wv", [D, LHD], BF16, kind="ExternalInput")
    wp_d = nc.dram_tensor("wp", [LHD, D], BF16, kind="ExternalInput")
    ones_d = nc.dram_tensor("ones", [1, 64], FP32R, kind="ExternalInput")
    bq_d = nc.dram_tensor("bq", [LHD, 1], FP32, kind="ExternalInput")
    bk_d = nc.dram_tensor("bk", [LHD, 1], FP32, kind="ExternalInput")
    bv_d = nc.dram_tensor("bv", [128, LHD], FP32, kind="ExternalInput")
    out_d = nc.dram_tensor("out", [S, D], FP16, kind="ExternalOutput")

    with tile.TileContext(nc) as tc:
        import contextlib

        with contextlib.ExitStack() as stk:
            # ---- long-lived SBUF pools --------------------------------
            qk_pool = stk.enter_context(tc.tile_pool(name="qk", bufs=1))
            v_pool = stk.enter_context(tc.tile_pool(name="v", bufs=1))
            ao_pool = stk.enter_context(tc.tile_pool(name="ao", bufs=1))
            const_pool = stk.enter_context(tc.tile_pool(name="const", bufs=1))
            w_pool = stk.enter_context(tc.tile_pool(name="w", bufs=1))
            x_pool = stk.enter_context(tc.tile_pool(name="x", bufs=1))

            qT = [qk_pool.tile([128, S], BF16, name=f"qT{p}", tag=f"qT{p}")
                  for p in range(2)]
            kT = [qk_pool.tile([128, S], BF16, name=f"kT{p}", tag=f"kT{p}")
                  for p in range(2)]
            # vp[p][m]: [128 keys, 130] = head2p v | 1.0 | head2p+1 v | 1.0
            vp = [[v_pool.tile([128, 130], BF16, name=f"v{p}_{m}",
                               tag=f"v{p}_{m}") for m in range(N_SK)]
                  for p in range(2)]
            aoT = [ao_pool.tile([128, S], BF16, name=f"ao{p}", tag=f"ao{p}")
                   for p in range(2)]

            ones_t = const_pool.tile([1, 64], FP32R, tag="ones")
            nc.sync.dma_start(ones_t[:], ones_d.ap())
            bq_t = [const_pool.tile([128, 1], FP32, name=f"bq{p}",
                                    tag=f"bq{p}") for p in range(2)]
            bk_t = [const_pool.tile([128, 1], FP32, name=f"bk{p}",
                                    tag=f"bk{p}") for p in range(2)]
            bv_t = const_pool.tile([128, LHD], FP32, tag="bv")
            for p in range(2):
                psl = slice(p * 128, (p + 1) * 128)
                nc.sync.dma_start(bq_t[p][:], bq_d.ap()[psl, :])
                nc.sync.dma_start(bk_t[p][:], bk_d.ap()[psl, :])
            nc.sync.dma_start(bv_t[:], bv_d.ap())

            x_t = [[x_pool.tile([128, SQ], BF16, name=f"x{k}_{sc}",
                                tag=f"x{k}_{sc}") for sc in range(4)]
                   for k in range(N_KT)]
            wq_t = [w_pool.tile([128, LHD], BF16, name=f"wq{k}", tag=f"wq{k}")
                    for k in range(N_KT)]
            wk_t = [w_pool.tile([128, LHD], BF16, name=f"wk{k}", tag=f"wk{k}")
                    for k in range(N_KT)]
            wv_t = [w_pool.tile([128, LHD], BF16, name=f"wv{k}", tag=f"wv{k}")
                    for k in range(N_KT)]
            wp_t = [w_pool.tile([128, D], BF16, name=f"wp{p}", tag=f"wp{p}")
                    for p in range(2)]

            # input DMA: x on sync queue, weights on scalar queue (scalar
            # engine is idle during phase A)
            def xdma(k, sc):
                sl = slice(k * 128, (k + 1) * 128)
                ssl = slice(sc * SQ, (sc + 1) * SQ)
                eng = nc.sync if k % 2 == 0 else nc.gpsimd
                eng.dma_start(x_t[k][sc][:], xT_d.ap()[sl, ssl])

            def wdma(w_t, w_d, k):
                sl = slice(k * 128, (k + 1) * 128)
                eng = nc.sync if k % 2 == 0 else nc.gpsimd
                eng.dma_start(w_t[k][:], w_d.ap()[sl, :])

            for k in range(N_KT):
                xdma(k, 0)
            for k in range(N_KT):
                wdma(wk_t, wk_d, k)
            for k in range(N_KT):
                wdma(wq_t, wq_d, k)
            for k in range(N_KT):
                nc.scalar.dma_start(wv_t[k][:],
                                    wv_d.ap()[k * 128:(k + 1) * 128, :])
            for sc in range(1, 4):
                for k in range(N_KT):
                    xdma(k, sc)
            for p in range(2):
                nc.scalar.dma_start(wp_t[p][:],
                                    wp_d.ap()[p * 128:(p + 1) * 128, :])

            # ---- phase B pools ----------------------------------------
            sc_pool = stk.enter_context(
                tc.tile_pool(name="sc", bufs=2, space="PSUM"))
            acc_pool = stk.enter_context(
                tc.tile_pool(name="acc", bufs=1, space="PSUM"))
            misc_pool = stk.enter_context(
                tc.tile_pool(name="misc", bufs=2, space="PSUM"))
            p_pool = stk.enter_context(tc.tile_pool(name="pt", bufs=6))
            rr_pool = stk.enter_context(tc.tile_pool(name="rr", bufs=2))
            rc_pool = stk.enter_context(tc.tile_pool(name="rc", bufs=2))
            ost_pool = stk.enter_context(tc.tile_pool(name="ost", bufs=2))

            # ---- phase-A chunk emitters (run through the misc ring) ----
            def qk0_chunk(which, sc):
                w_t, b_t, dstT = ((wq_t, bq_t[0], qT[0]) if which == "q"
                                  else (wk_t, bk_t[0], kT[0]))
                ssl = slice(sc * SQ, (sc + 1) * SQ)

                def emit():
                    ps = misc_pool.tile([128, SQ], FP32,
                                        name=f"m0{which}{sc}", tag="m")
                    for k in range(N_KT):
                        nc.tensor.matmul(
                            ps[:], w_t[k][:, 0:128], x_t[k][sc][:],
                            start=(k == 0), stop=(k == N_KT - 1),
                        )
                    nc.vector.tensor_scalar(
                        dstT[:, ssl], ps[:], b_t[:], None,
                        mybir.AluOpType.add,
                    )
                return emit

            def v_chunk(m):
                def emit():
                    ps = misc_pool.tile([128, SQ], FP32,
                                        name=f"m0v{m}", tag="m")
                    xo = (m % 4) * 128
                    for k in range(N_KT):
                        nc.tensor.matmul(
                            ps[:, 0:LHD],
                            x_t[k][m // 4][:, xo:xo + 128], wv_t[k][:],
                            start=(k == 0), stop=(k == N_KT - 1),
                        )
                    for p in range(2):
                        nc.vector.tensor_tensor(
                            vp[p][m][:, 0:64], ps[:, p * 128:p * 128 + 64],
                            bv_t[:, p * 128:p * 128 + 64],
                            mybir.AluOpType.add,
                        )
                        nc.vector.tensor_tensor(
                            vp[p][m][:, 65:129],
                            ps[:, p * 128 + 64:p * 128 + 128],
                            bv_t[:, p * 128 + 64:p * 128 + 128],
                            mybir.AluOpType.add,
                        )
                        nc.vector.memset(vp[p][m][:, 64::65], 1.0)
                return emit

            # ---- deferred emission units (PE filler work) -------------
            filler = []

            def qk1_chunk_parts(which, sc):
                # q/k pair-1 chunk: 8 accumulating MMs + DVE drain, split
                # into 2-MM pieces so injection granularity stays ~0.5us
                w_t, b_t, dstT = ((wq_t, bq_t[1], qT[1]) if which == "q"
                                  else (wk_t, bk_t[1], kT[1]))
                ssl = slice(sc * SQ, (sc + 1) * SQ)
                state = {}

                def piece(k0):
                    def emit():
                        if k0 == 0:
                            state["ps"] = misc_pool.tile(
                                [128, SQ], FP32,
                                name=f"mqk{which}{sc}", tag="m")
                        ps = state["ps"]
                        for k in (k0, k0 + 1):
                            nc.tensor.matmul(
                                ps[:], w_t[k][:, 128:256], x_t[k][sc][:],
                                start=(k == 0), stop=(k == N_KT - 1),
                            )
                        if k0 == N_KT - 2:
                            nc.vector.tensor_scalar(
                                dstT[:, ssl], ps[:], b_t[:], None,
                                mybir.AluOpType.add,
                            )
                    return emit
                return [piece(k0) for k0 in range(0, N_KT, 2)]

            for sc in range(4):
                filler.extend(qk1_chunk_parts("q", sc))
                filler.extend(qk1_chunk_parts("k", sc))

            def proj_unit(m, nb, tail=False):
                # partial projection for s-tile m, dout half nb
                msl = slice(m * 128, (m + 1) * 128)
                nsl = slice(nb * SQ, (nb + 1) * SQ)

                def emit():
                    if tail and (m + nb) % 2 == 0:
                        big = sc_pool.tile([128, 2 * SQ], FP32,
                                           name=f"spj{m}_{nb}", tag="sc")
                        ps = big[:, 0:SQ]
                    else:
                        ps = misc_pool.tile([128, SQ], FP32,
                                            name=f"mpj{m}_{nb}", tag="m")
                    for p in range(2):
                        nc.tensor.matmul(
                            ps[:], aoT[p][:, msl], wp_t[p][:, nsl],
                            start=(p == 0), stop=(p == 1),
                        )
                    ot = ost_pool.tile([128, SQ], FP16,
                                       name=f"ot{m}_{nb}", tag="ot")
                    if tail and (m + nb) % 2 == 0:
                        nc.scalar.copy(ot[:], ps[:])
                    else:
                        nc.vector.tensor_copy(ot[:], ps[:])
                    eng = nc.gpsimd if (m + nb) % 2 == 0 else nc.sync
                    eng.dma_start(out_d.ap()[msl, nsl], ot[:])
                return emit

            def inject(n):
                for _ in range(n):
                    if filler:
                        filler.pop(0)()

            # ---- phase B: attention -----------------------------------
            for p in range(2):
                for cq in range(N_CQ):
                    qsl = slice(cq * SQ, (cq + 1) * SQ)
                    acc_a = acc_pool.tile([65, SQ], FP32, tag="acca")
                    acc_b = acc_pool.tile([65, SQ], FP32, tag="accb")
                    for t in range(N_SK):
                        if p == 0 and cq == 0 and t % 4 == 0:
                            # qk(p0) for key/query column chunk t//4,
                            # gated on x pieces as they stream in
                            sc = t // 4
                            qk0_chunk("k", sc)()
                            if sc == 0:
                                qk0_chunk("q", sc)()
                        tsl = slice(t * 128, (t + 1) * 128)
                        sc_ab = sc_pool.tile([128, 2 * SQ], FP32, tag="sc")
                        nc.tensor.matmul(
                            sc_ab[:, 0:SQ], kT[p][0:64, tsl],
                            qT[p][0:64, qsl],
                            start=True, stop=True, tile_position=(0, 0),
                        )
                        nc.tensor.matmul(
                            sc_ab[:, SQ:2 * SQ], kT[p][64:128, tsl],
                            qT[p][64:128, qsl],
                            start=True, stop=True, tile_position=(64, 0),
                        )
                        pab = p_pool.tile([128, 2 * SQ], BF16, tag="pt")
                        nc.scalar.activation(
                            pab[:], sc_ab[:],
                            mybir.ActivationFunctionType.Exp, scale=0.125,
                        )
                        if p == 0 and cq == 0:
                            v_chunk(t)()
                            if t % 4 == 3 and t < 12:
                                qk0_chunk("q", t // 4 + 1)()
                        nc.tensor.matmul(
                            acc_a[:], vp[p][t][:, 0:65], pab[:, 0:SQ],
                            start=(t == 0), stop=(t == N_SK - 1),
                        )
                        nc.tensor.matmul(
                            acc_b[:], vp[p][t][:, 65:130], pab[:, SQ:2 * SQ],
                            start=(t == 0), stop=(t == N_SK - 1),
                        )
                        if not (p == 0 and cq == 0):
                            inject(1)
                    # normalize both heads of this (p, cq)
                    for acc, half in ((acc_a, 0), (acc_b, 1)):
                        rrow = rr_pool.tile([1, SQ], FP32R, tag="rrow")
                        nc.vector.tensor_copy(rrow[:], acc[64:65, :])
                        rbt = misc_pool.tile([128, SQ], FP32, tag="m",
                                             name=f"rb{p}{cq}{half}")
                        nc.tensor.matmul(
                            rbt[0:64, :], ones_t[0:1, 0:64], rrow[:],
                            start=True, stop=True,
                        )
                        rc = rc_pool.tile([64, SQ], FP32, tag="rc")
                        nc.vector.reciprocal_approx_fast(rc[:], rbt[0:64, :])
                        nc.vector.tensor_tensor(
                            aoT[p][64 * half:64 * half + 64, qsl],
                            acc[0:64, :], rc[:],
                            mybir.AluOpType.mult,
                        )
                    if p == 1:
                        # projection for the chunk finished one cq ago
                        # rides inside the next chunk's t-loop via filler
                        tail = cq == N_CQ - 1
                        for m in range(cq * 4, cq * 4 + 4):
                            for nb in range(2):
                                filler.append(proj_unit(m, nb, tail))
            # flush remaining filler (last chunk's projection)
            while filler:
                filler.pop(0)()

    nc.compile()
    return nc


def _get_program():
    global _compiled
    if _compiled is None:
        _compiled = _build()
    return _compiled


def _make_in_maps(x, w_qkv, b_qkv, w_proj, b_proj):
    x = np.asarray(x, dtype=np.float32)
    w_qkv = np.asarray(w_qkv, dtype=np.float32)
    b_qkv = np.asarray(b_qkv, dtype=np.float32)
    w_proj = np.asarray(w_proj, dtype=np.float32)

    in_maps = []
    for c in range(N_CORES):
        g, r = c // GROUP, c % GROUP
        xT = np.ascontiguousarray(x[g].T)
        csl = slice(r * LHD, (r + 1) * LHD)
        in_maps.append(
            {
                "xT": xT.astype(BF16_NP),
                "wq": w_qkv[:, 0 * D + r * LHD:0 * D + (r + 1) * LHD].astype(BF16_NP),
                "wk": w_qkv[:, 1 * D + r * LHD:1 * D + (r + 1) * LHD].astype(BF16_NP),
                "wv": w_qkv[:, 2 * D + r * LHD:2 * D + (r + 1) * LHD].astype(BF16_NP),
                "wp": np.ascontiguousarray(w_proj[csl, :]).astype(BF16_NP),
                "bq": np.ascontiguousarray(
                    b_qkv[0 * D + r * LHD:0 * D + (r + 1) * LHD].reshape(LHD, 1)),
                "bk": np.ascontiguousarray(
                    b_qkv[1 * D + r * LHD:1 * D + (r + 1) * LHD].reshape(LHD, 1)),
                "ones": _ONES,
                "bv": np.ascontiguousarray(
                    np.broadcast_to(
                        b_qkv[2 * D + r * LHD:2 * D + (r + 1) * LHD].reshape(1, LHD),
                        (128, LHD),
                    )
                ),
            }
        )
    return in_maps


def _assemble(results, b_proj):
    out = np.empty((B, S, D), dtype=np.float32)
    for g in range(B):
        acc = results[g * GROUP]["out"].astype(np.float32)
        for r in range(1, GROUP):
            acc += results[g * GROUP + r]["out"].astype(np.float32)
        out[g] = acc + np.asarray(b_proj, dtype=np.float32).reshape(1, D)
    return out


def kernel(x, w_qkv, b_qkv, w_proj, b_proj):
    nc = _get_program()
    in_maps = _make_in_maps(x, w_qkv, b_qkv, w_proj, b_proj)
    res = run_bass_kernel_spmd(nc, in_maps, list(range(N_CORES)))
    return _assemble(res.results, b_proj)
